# revision 15
# baseline (speedup 1.0000x reference)
"""Trainium2 Bass kernel for nn_CausalSelfAttention_68496138437292.

Sharding: 8 cores = 2 batches x 4 head-groups. Core c handles batch b=c//4 and
heads [4*(c%4), 4*(c%4)+4). The Tversky projection is sharded over out_features
(each core computes a 256-wide o-slice); the feature contraction x_f (summed
over D, which is split across head-groups) uses one small AllReduce over each
batch's 4-core group.

All matmuls run in float32r (full-rate fp32 PE mode). The ternary weight
quantization (bf16 group-wise, matching the reference bit-exactly) and the
RoPE/YaRN tables are precomputed on the host; all heavy math (QKV projection,
rmsnorm stats, rope rotation, causal attention, softmax, Tversky projection)
runs on device.

Layout notes:
- q/k are computed transposed (d on partitions, s free). Matmul operands must
  start at base partition 0/32/64 and DVE ops cannot cross partitions, so the
  8 per-(head,half) 32-row blocks are packed 3-per-tile at offsets {0,32,64};
  projection PSUM is drained by DMA straight into that block layout.
- v lands [s, channel] groups of 33 (32 v columns + a ones column) so the PV
  matmul emits softmax denominators for free in PSUM row 32.
- Scores are computed transposed (S^T[ks, qs]); exp's per-partition ACT scale
  applies the k-side rmsnorm factor, the q-side factor (with q_gain/sqrt(d))
  is multiplied into q after rope via a DMA-broadcast tile.
- All partition-crossing moves (rope half-swap staging, reciprocal broadcast,
  attention-output writeback) go through DMA.
"""

import math
from contextlib import ExitStack

import ml_dtypes
import numpy as np

import concourse.bass as bass
import concourse.mybir as mybir
import concourse.tile as tile
from concourse import bacc
from concourse.bass_utils import run_bass_kernel_spmd

F32 = mybir.dt.float32
F32R = mybir.dt.float32r
AF = mybir.ActivationFunctionType
ALU = mybir.AluOpType

DIM, NH, HD = 1024, 16, 64
ROPE_BASE, TRAIN_LEN, YARN_MAX = 10000.0, 1024, 4096
GROUP = 64
EPS = 1e-05
B = 2
N_CORES = 8
HPC = 4          # heads per core
OSL = 256        # out-feature slice per core


# block layout: (head, half) -> (tile, partition offset); 3 blocks per tile
def _blk(h, f):
    idx = h * 2 + f
    return idx // 3, (idx % 3) * 32


_NBLK = [3, 3, 2]                      # valid blocks per q/k tile
_NROW = [96, 96, 64]                   # valid rows per q/k tile


# ----------------------------------------------------------------- host math

def _ternary_deq(w: np.ndarray) -> np.ndarray:
    """bf16 group-wise ternary dequantized weights; bit-exact with the jax
    reference (mean accumulates in f32, every elementwise op rounds to bf16)."""
    bf = ml_dtypes.bfloat16
    wb = w.astype(bf)
    wg = wb.reshape(-1, GROUP)
    m = (np.sum(np.abs(wg), axis=-1, keepdims=True, dtype=np.float32) / GROUP).astype(bf)
    scale = np.maximum(m.astype(np.float32), np.float32(1e-8)).astype(bf)
    ratio = (wg.astype(np.float32) / scale.astype(np.float32)).astype(bf)
    q = np.clip(np.round(ratio.astype(np.float32)), -1.0, 1.0).astype(bf)
    deq = (q.astype(np.float32) * scale.astype(np.float32)).astype(bf)
    return deq.reshape(wb.shape).astype(np.float32)


def _rope_tables(seqlen: int):
    rd = HD
    ar = np.arange(0, rd, 2, dtype=np.float32)
    inv_freq = 1.0 / ROPE_BASE ** (ar / rd)
    scale = TRAIN_LEN / YARN_MAX
    ramp = np.clip((ar / rd - 0.25) / 0.75, 0.0, 1.0)
    inv_freq = inv_freq / (ramp * (1.0 / scale - 1.0) + 1.0)
    freqs = np.arange(seqlen, dtype=np.float32)[:, None] * inv_freq[None, :]
    # [S, 32] -> [32, S]
    return np.cos(freqs).T.astype(np.float32), np.sin(freqs).T.astype(np.float32)


def _sigmoid(x):
    return 1.0 / (1.0 + np.exp(-x))


# ------------------------------------------------------------ device program

def build_program(S: int, lam: np.ndarray, dbg: bool = False):
    """Build the SPMD Bass program. lam: [HPC] per-local-head diff_lambda
    (baked as immediates; must be identical across head groups)."""
    NT = S // 128          # s-tiles
    NJ = S // 512          # 512-wide qs chunks
    SQ = S // 4            # projection s-quarter width
    NTQ = SQ // 128        # s-tiles per quarter

    nc = bacc.Bacc("TRN2", target_bir_lowering=False, debug=False,
                   num_devices=N_CORES)

    # DRAM I/O (per-core contents differ via in_maps)
    d_xT = nc.dram_tensor("xT", [DIM, S], F32R, kind="ExternalInput")
    d_wqkT = nc.dram_tensor("wqkT", [DIM, 512], F32R, kind="ExternalInput")
    d_wvT = nc.dram_tensor("wvT", [DIM, 256], F32R, kind="ExternalInput")
    d_cosF = nc.dram_tensor("cosF", [128, S], F32, kind="ExternalInput")
    d_sinF = nc.dram_tensor("sinF", [128, S], F32, kind="ExternalInput")
    d_bigmask = nc.dram_tensor("bigmask", [128, 896], F32, kind="ExternalInput")
    d_maskq = [nc.dram_tensor(f"maskq{t}", [128, 4], F32R, kind="ExternalInput")
               for t in range(3)]
    d_maskk = [nc.dram_tensor(f"maskk{t}", [128, 4], F32R, kind="ExternalInput")
               for t in range(3)]
    d_gains = nc.dram_tensor("gains", [4, 1], F32, kind="ExternalInput")
    d_featT = nc.dram_tensor("featT", [OSL, 16], F32R, kind="ExternalInput")
    d_AT = nc.dram_tensor("AT", [16, OSL], F32R, kind="ExternalInput")
    d_BT = nc.dram_tensor("BT", [16, OSL], F32R, kind="ExternalInput")
    d_out = nc.dram_tensor("out", [OSL, S], F32, kind="ExternalOutput")
    if dbg:
        d_dbg_qa = [nc.dram_tensor(f"dbg_qa{t}", [128, S], F32, kind="ExternalOutput")
                    for t in range(3)]
        d_dbg_ka = [nc.dram_tensor(f"dbg_ka{t}", [128, S], F32, kind="ExternalOutput")
                    for t in range(3)]
        d_dbg_v = nc.dram_tensor("dbg_v0", [128, 264], F32, kind="ExternalOutput")
        d_dbg_invT = nc.dram_tensor("dbg_invT", [128, 4 * (S // 128)], F32,
                                    kind="ExternalOutput")
        d_dbg_inva = nc.dram_tensor("dbg_inva", [4, S], F32, kind="ExternalOutput")
        d_dbg_yT = [nc.dram_tensor(f"dbg_yT{i}", [128, S], F32, kind="ExternalOutput")
                    for i in range(2)]
        d_dbg_xf = nc.dram_tensor("dbg_xf", [16, S], F32, kind="ExternalOutput")

    with tile.TileContext(nc) as tc:
        persist = ExitStack()
        cpool = persist.enter_context(tc.tile_pool(name="consts", bufs=1))
        vpool = persist.enter_context(tc.tile_pool(name="vstore", bufs=1))
        ivpool = persist.enter_context(tc.tile_pool(name="invT", bufs=1))
        ypool = persist.enter_context(tc.tile_pool(name="yT", bufs=1))
        qkpool = persist.enter_context(tc.tile_pool(name="qk", bufs=1))
        drpool = persist.enter_context(
            tc.tile_pool(name="drscratch", bufs=1, space="DRAM"))

        # small constants (live for the whole kernel)
        bigmask = cpool.tile([128, 896], F32, name="bigmask")
        maskq = [cpool.tile([128, 4], F32R, name=f"maskq{t}") for t in range(3)]
        maskk = [cpool.tile([128, 4], F32R, name=f"maskk{t}") for t in range(3)]
        gains = cpool.tile([4, 1], F32, name="gains")
        eps_col = cpool.tile([128, 1], F32, name="eps")
        featT = [cpool.tile([128, 16], F32R, name=f"featT{i}") for i in range(2)]
        AT = cpool.tile([16, OSL], F32R, name="AT")
        BT = cpool.tile([16, OSL], F32R, name="BT")
        nc.vector.memset(eps_col[:], EPS)
        nc.sync.dma_start(bigmask[:], d_bigmask[:])
        for t in range(3):
            nc.sync.dma_start(maskq[t][:], d_maskq[t][:])
            nc.sync.dma_start(maskk[t][:], d_maskk[t][:])
        nc.sync.dma_start(gains[:], d_gains[:])
        nc.sync.dma_start(featT[0][:], d_featT[0:128, :])
        nc.sync.dma_start(featT[1][:], d_featT[128:256, :])
        nc.sync.dma_start(AT[:], d_AT[:])
        nc.sync.dma_start(BT[:], d_BT[:])

        # v storage: per s-tile, 8 groups of (32 v-cols + ones col)
        v_store = [vpool.tile([128, 264], F32R, name=f"v{st}") for st in range(NT)]
        invT = ivpool.tile([128, 4 * NT], F32, name="invT")
        yT = [ypool.tile([128, S], F32R, name=f"yT{i}") for i in range(2)]
        # q/k in block layout; rope happens in place on these tiles
        qa = [qkpool.tile([128, S], F32R, name=f"qa{t}") for t in range(3)]
        ka = [qkpool.tile([128, S], F32R, name=f"ka{t}") for t in range(3)]

        # ---------------- phase 1: QKV projection ----------------
        proj_scope = ExitStack()
        wpool = proj_scope.enter_context(tc.tile_pool(name="weights", bufs=1))
        xpool = proj_scope.enter_context(tc.tile_pool(name="xstream", bufs=3))
        psq = proj_scope.enter_context(
            tc.tile_pool(name="psq", bufs=1, space="PSUM"))
        psv = proj_scope.enter_context(
            tc.tile_pool(name="psv", bufs=1, space="PSUM"))

        wqk = [wpool.tile([128, 512], F32R, name=f"wqk{d}") for d in range(8)]
        wv = [wpool.tile([128, 256], F32R, name=f"wv{d}") for d in range(8)]
        for d in range(8):
            nc.sync.dma_start(wqk[d][:], d_wqkT[d * 128:(d + 1) * 128, :])
            nc.sync.dma_start(wv[d][:], d_wvT[d * 128:(d + 1) * 128, :])

        for q4 in range(4):
            s0 = q4 * SQ
            pq = [psq.tile([128, SQ], F32, tag=f"pq{ch}", name=f"pq{ch}")
                  for ch in range(4)]
            pv = [psv.tile([128, 256], F32, tag=f"pv{st}", name=f"pv{st}")
                  for st in range(NTQ)]
            for d in range(8):
                xt = xpool.tile([128, SQ], F32R, tag="xt", name="xt")
                nc.sync.dma_start(xt[:], d_xT[d * 128:(d + 1) * 128, s0:s0 + SQ])
                for ch in range(4):
                    nc.tensor.matmul(
                        pq[ch][:], wqk[d][:, ch * 128:(ch + 1) * 128], xt[:],
                        start=(d == 0), stop=(d == 7))
                for st in range(NTQ):
                    nc.tensor.matmul(
                        pv[st][:], xt[:, st * 128:(st + 1) * 128], wv[d][:],
                        start=(d == 0), stop=(d == 7))
            # qk drain: ACT copy psum -> aligned SBUF temp, then DMA each
            # (head, half) 32-row block into the packed block layout.
            # psum ch-tile layout: 2 heads x 64.
            for ch in range(4):
                tmpd = xpool.tile([128, SQ], F32R, tag="tmpd", name="tmpd",
                                  bufs=4)
                nc.scalar.activation(tmpd[:], pq[ch][:], AF.Copy)
                dst_tiles = qa if ch < 2 else ka
                for hl in range(2):          # head within ch-tile
                    h = (ch % 2) * 2 + hl
                    for f in range(2):
                        t, o = _blk(h, f)
                        nc.sync.dma_start(
                            dst_tiles[t][o:o + 32, s0:s0 + SQ],
                            tmpd[hl * 64 + f * 32:hl * 64 + f * 32 + 32, :])
            for st in range(NTQ):
                gst = q4 * NTQ + st
                nc.vector.memset(v_store[gst][:].bitcast(F32), 1.0)
                dst = v_store[gst][:].rearrange("p (g c) -> p g c", c=33)[:, :, 0:32]
                src = pv[st][:].rearrange("p (g c) -> p g c", c=32)
                nc.scalar.activation(dst, src, AF.Copy)
        proj_scope.close()

        # ---------------- phase 2: rmsnorm stats ----------------
        qsc_scope = ExitStack()
        qsc_pool = qsc_scope.enter_context(tc.tile_pool(name="qscale", bufs=1))
        norm_scope = ExitStack()
        sqpool = norm_scope.enter_context(tc.tile_pool(name="sq", bufs=1))
        inva_pool = norm_scope.enter_context(tc.tile_pool(name="inva", bufs=1))
        psa = norm_scope.enter_context(
            tc.tile_pool(name="psa", bufs=1, space="PSUM"))
        psb = norm_scope.enter_context(
            tc.tile_pool(name="psb", bufs=3, space="PSUM"))

        # q-side: sumsq per (head, s) as [4, S]
        pa = psa.tile([4, S], F32, name="pa")
        for t in range(3):
            n = _NROW[t]
            sq = sqpool.tile([128, S], F32R, tag=f"sq{t}", name=f"sq{t}")
            nc.scalar.activation(sq[0:n, :], qa[t][0:n, :], AF.Square)
            for qc in range(S // 512):
                nc.tensor.matmul(
                    pa[:, qc * 512:(qc + 1) * 512], maskq[t][0:n, :],
                    sq[0:n, qc * 512:(qc + 1) * 512],
                    start=(t == 0), stop=(t == 2))
        inv_a = inva_pool.tile([4, S], F32, name="inva")
        nc.scalar.activation(inv_a[:], pa[:], AF.Sqrt, scale=1.0 / HD,
                             bias=eps_col[0:4, :])
        nc.vector.reciprocal(inv_a[:], inv_a[:])
        nc.vector.tensor_scalar_mul(inv_a[:], inv_a[:], gains[:])

        if dbg:
            nc.sync.dma_start(d_dbg_inva[:], inv_a[:])
        # broadcast inv_a rows into q-block layout via DRAM round trip
        dr_inva = drpool.tile([4, S], F32, name="dr_inva")
        nc.sync.dma_start(dr_inva[:], inv_a[:])
        qsc = [qsc_pool.tile([128, S], F32, name=f"qsc{t}") for t in range(3)]
        for h in range(4):
            for f in range(2):
                t, o = _blk(h, f)
                nc.sync.dma_start(
                    qsc[t][o:o + 32, :],
                    dr_inva[h:h + 1, :].to_broadcast([32, S]))

        # k-side: sumsq transposed as [s-tile, 4] columns of invT
        sq_k = []
        for t in range(3):
            n = _NROW[t]
            sq = sqpool.tile([128, S], F32R, tag=f"sqk{t}", name=f"sqk{t}")
            nc.scalar.activation(sq[0:n, :], ka[t][0:n, :], AF.Square)
            sq_k.append(sq)
        for st in range(NT):
            pb = psb.tile([128, 4], F32, tag="pb", name="pb")
            for t in range(3):
                n = _NROW[t]
                nc.tensor.matmul(
                    pb[:], sq_k[t][0:n, st * 128:(st + 1) * 128], maskk[t][0:n, :],
                    start=(t == 0), stop=(t == 2))
            nc.scalar.activation(invT[:, st * 4:st * 4 + 4], pb[:], AF.Sqrt,
                                 scale=1.0 / HD, bias=eps_col[:])
        nc.vector.reciprocal(invT[:], invT[:])
        norm_scope.close()

        # -------------- phase 3: rope (+ q scaling), in place --------------
        rope_scope = ExitStack()
        tripool = rope_scope.enter_context(tc.tile_pool(name="trig", bufs=1))
        rp = rope_scope.enter_context(tc.tile_pool(name="ropetmp", bufs=2))

        cosF = tripool.tile([128, S], F32, name="cosF")
        sinF = tripool.tile([128, S], F32, name="sinF")
        nc.sync.dma_start(cosF[:], d_cosF[:])
        nc.sync.dma_start(sinF[:], d_sinF[:])

        def rope_all(tiles, qscale_tiles):
            # Prefetch ALL partner copies (other half of each head) before the
            # in-place overwrites: partner pairs can span tiles (h1's halves
            # live in tiles 0 and 1).
            prts = []
            for t in range(3):
                prt = rp.tile([128, S], F32R, tag=f"prt{t}", name=f"prt{t}")
                for k in range(_NBLK[t]):
                    idx = 3 * t + k
                    h, f = idx // 2, idx % 2
                    pt, po = _blk(h, 1 - f)
                    nc.sync.dma_start(prt[k * 32:k * 32 + 32, :],
                                      tiles[pt][po:po + 32, :])
                prts.append(prt)
            # rot_f0 = x0*cos + x1*sin ; rot_f1 = x1*cos - x0*sin.
            # sinF rows carry sign (+,-,+) per 32-block, matching the f-parity
            # of blocks in tiles 0 and 2; tile 1 has parity (-,+,-) so the
            # sign flips there -> subtract instead of add.
            for t in range(3):
                n = _NROW[t]
                tb = rp.tile([128, S], F32, tag="tb", name="tb")
                nc.vector.tensor_mul(tb[0:n, :], prts[t][0:n, :], sinF[0:n, :])
                nc.vector.tensor_mul(tiles[t][0:n, :], tiles[t][0:n, :],
                                     cosF[0:n, :])
                if t == 1:
                    nc.vector.tensor_sub(tiles[t][0:n, :], tiles[t][0:n, :],
                                         tb[0:n, :])
                else:
                    nc.vector.tensor_add(tiles[t][0:n, :], tiles[t][0:n, :],
                                         tb[0:n, :])
                if qscale_tiles is not None:
                    nc.vector.tensor_mul(tiles[t][0:n, :], tiles[t][0:n, :],
                                         qscale_tiles[t][0:n, :])

        rope_all(qa, qsc)
        rope_all(ka, None)
        rope_scope.close()
        qsc_scope.close()
        if dbg:
            for t in range(3):
                nc.sync.dma_start(d_dbg_qa[t][:], qa[t][:].bitcast(F32))
                nc.sync.dma_start(d_dbg_ka[t][:], ka[t][:].bitcast(F32))
            nc.sync.dma_start(d_dbg_v[:], v_store[0][:].bitcast(F32))
            nc.sync.dma_start(d_dbg_invT[:], invT[:])

        # ---------------- phase 4: attention ----------------
        attn_scope = ExitStack()
        epool = attn_scope.enter_context(tc.tile_pool(name="expS", bufs=18))
        tpool = attn_scope.enter_context(tc.tile_pool(name="exptmp", bufs=4))
        zpool = attn_scope.enter_context(tc.tile_pool(name="zc", bufs=6))
        rcpool = attn_scope.enter_context(tc.tile_pool(name="rcp", bufs=4))
        bcpool = attn_scope.enter_context(tc.tile_pool(name="bcast", bufs=4))
        pss = attn_scope.enter_context(
            tc.tile_pool(name="pss", bufs=4, space="PSUM"))
        psy = attn_scope.enter_context(
            tc.tile_pool(name="psy", bufs=4, space="PSUM"))

        for h in range(HPC):
            for j in range(NJ):
                ntk = 4 * (j + 1)
                es = {}
                for f in (0, 1):
                    qt, qo = _blk(h, f)
                    qr = qa[qt][qo:qo + 32, j * 512:(j + 1) * 512]
                    for t in range(ntk):
                        ps = pss.tile([128, 512], F32, tag="ps", name="ps")
                        nc.tensor.matmul(
                            ps[:],
                            ka[qt][qo:qo + 32, t * 128:(t + 1) * 128],
                            qr, start=True, stop=True)
                        sc = invT[:, t * 4 + h:t * 4 + h + 1]
                        et = epool.tile([128, 512], F32R, tag="e", name="e")
                        if t >= 4 * j:  # causal boundary tile
                            off = (t - 4 * j) * 128
                            tmp = tpool.tile([128, 512], F32, tag="tmp",
                                             name="tmp")
                            nc.scalar.activation(tmp[:], ps[:], AF.Exp, scale=sc)
                            nc.vector.tensor_mul(
                                et[:], tmp[:],
                                bigmask[:, 384 - off:896 - off])
                        else:
                            nc.scalar.activation(et[:], ps[:], AF.Exp, scale=sc)
                        es[(f, t)] = et
                py = {}
                for f in (0, 1):
                    ps_y = psy.tile([33, 512], F32, tag="py", name="py")
                    for t in range(ntk):
                        g = 2 * h + f
                        nc.tensor.matmul(
                            ps_y[:], v_store[t][:, g * 33:g * 33 + 33],
                            es[(f, t)][:], start=(t == 0), stop=(t == ntk - 1))
                    py[f] = ps_y
                # combine halves (all at base partition 0, then DMA into yT)
                bb = {}
                for f in (0, 1):
                    rc = rcpool.tile([33, 512], F32, tag="rc", name="rc")
                    nc.vector.reciprocal(rc[32:33, :], py[f][32:33, :])
                    dr_rc = drpool.tile([1, 512], F32, tag="drrc", bufs=4,
                                        name="drrc")
                    nc.sync.dma_start(dr_rc[:], rc[32:33, :])
                    bc = bcpool.tile([32, 512], F32, tag="bc", name="bc")
                    nc.sync.dma_start(bc[:], dr_rc[:].to_broadcast([32, 512]))
                    bb[f] = bc
                z1 = zpool.tile([32, 512], F32, tag="z", name="z")
                z2 = zpool.tile([32, 512], F32, tag="z", name="z")
                zo1 = zpool.tile([32, 512], F32, tag="z", name="z")
                zo2 = zpool.tile([32, 512], F32, tag="z", name="z")
                nc.vector.tensor_mul(z1[:], py[0][0:32, :], bb[0][:])
                nc.vector.tensor_mul(z2[:], py[1][0:32, :], bb[1][:])
                lam_h = float(lam[h])
                nc.vector.scalar_tensor_tensor(
                    zo1[:], z2[:], -lam_h, z1[:], ALU.mult, ALU.add)
                nc.vector.scalar_tensor_tensor(
                    zo2[:], z2[:], lam_h, z1[:], ALU.mult, ALU.add)
                ti, r0 = h // 2, (h % 2) * 64
                nc.sync.dma_start(
                    yT[ti][r0:r0 + 32, j * 512:(j + 1) * 512],
                    zo1[:].bitcast(F32R))
                nc.sync.dma_start(
                    yT[ti][r0 + 32:r0 + 64, j * 512:(j + 1) * 512],
                    zo2[:].bitcast(F32R))
        attn_scope.close()

        # ---------------- phase 5: tversky projection ----------------
        tv_scope = ExitStack()
        xfpool = tv_scope.enter_context(tc.tile_pool(name="xf", bufs=1))
        psxf = tv_scope.enter_context(
            tc.tile_pool(name="psxf", bufs=1, space="PSUM"))

        pxf = psxf.tile([16, S], F32, name="pxf")
        for qc in range(S // 512):
            for dc in range(2):
                nc.tensor.matmul(
                    pxf[:, qc * 512:(qc + 1) * 512], featT[dc][:],
                    yT[dc][:, qc * 512:(qc + 1) * 512],
                    start=(dc == 0), stop=(dc == 1))
        xf_loc = xfpool.tile([16, S], F32, name="xfl")
        nc.scalar.activation(xf_loc[:], pxf[:], AF.Copy)

        cc_in = drpool.tile([16, S], F32, name="ccin")
        cc_out = drpool.tile([16, S], F32, name="ccout")
        nc.sync.dma_start(cc_in[:], xf_loc[:])
        nc.gpsimd.collective_compute(
            "AllReduce", ALU.add,
            replica_groups=[[0, 1, 2, 3], [4, 5, 6, 7]],
            ins=[cc_in[:]], outs=[cc_out[:]])
        xf = xfpool.tile([16, S], F32, name="xfr")
        nc.sync.dma_start(xf[:], cc_out[:])
        if dbg:
            for i in range(2):
                nc.sync.dma_start(d_dbg_yT[i][:], yT[i][:].bitcast(F32))
            nc.sync.dma_start(d_dbg_xf[:], xf[:])

        xa = xfpool.tile([16, S], F32R, name="xa")
        oms = xfpool.tile([16, S], F32R, name="oms")
        nc.scalar.activation(xa[:], xf[:], AF.Silu, scale=5.0)
        nc.scalar.activation(oms[:], xf[:], AF.Sigmoid, scale=-5.0)

        tv2_scope = ExitStack()
        opool = tv2_scope.enter_context(tc.tile_pool(name="outsb", bufs=1))
        pso = tv2_scope.enter_context(
            tc.tile_pool(name="pso", bufs=2, space="PSUM"))

        out_sb = [opool.tile([128, S], F32, name=f"osb{i}") for i in range(2)]
        for ot in range(2):
            for qc in range(S // 512):
                po = pso.tile([128, 512], F32, tag="po", name="po")
                nc.tensor.matmul(
                    po[:], AT[:, ot * 128:(ot + 1) * 128],
                    xa[:, qc * 512:(qc + 1) * 512], start=True, stop=False)
                nc.tensor.matmul(
                    po[:], BT[:, ot * 128:(ot + 1) * 128],
                    oms[:, qc * 512:(qc + 1) * 512], start=False, stop=True)
                nc.scalar.activation(
                    out_sb[ot][:, qc * 512:(qc + 1) * 512], po[:], AF.Copy)
            nc.sync.dma_start(d_out[ot * 128:(ot + 1) * 128, :], out_sb[ot][:])
        tv2_scope.close()
        tv_scope.close()
        persist.close()

    nc.compile()
    return nc


# ----------------------------------------------------------- host marshaling

def make_in_maps(S, x, w_qkv, features, prototypes, theta, alpha, beta,
                 q_gain, diff_lambda):
    x = np.asarray(x, np.float32)
    w_qkv = np.asarray(w_qkv, np.float32)
    features = np.asarray(features, np.float32)
    prototypes = np.asarray(prototypes, np.float32)
    theta = float(np.abs(np.asarray(theta, np.float32)))
    alpha = float(np.abs(np.asarray(alpha, np.float32)))
    beta = float(np.abs(np.asarray(beta, np.float32)))
    q_gain = np.asarray(q_gain, np.float32)

    w_deq = _ternary_deq(w_qkv)
    p_deq = _ternary_deq(prototypes)
    cosT, sinT = _rope_tables(S)       # [32, S] each

    rows = np.arange(128)
    # rope tables in block layout: row r belongs to block r//32 with d = r%32.
    # sinF carries the sign of the f-parity pattern of tiles 0/2 (+,-,+);
    # tile 1's pattern (-,+,-) is realized by subtracting instead of adding.
    cosF = cosT[rows % 32, :]
    sgn = np.where((rows // 32) % 2 == 0, 1.0, -1.0).astype(np.float32)
    sinF = sinT[rows % 32, :] * sgn[:, None]

    bigmask = (np.arange(896)[None, :] >= rows[:, None] + 384).astype(np.float32)

    # norm masks in block layout: block k of tile t belongs to head (3t+k)//2
    masks = []
    for t in range(3):
        m = np.zeros((128, 4), np.float32)
        for k in range(_NBLK[t]):
            head = (3 * t + k) // 2
            m[k * 32:(k + 1) * 32, head] = 1.0
        masks.append(m)

    in_maps = []
    for c in range(N_CORES):
        b, hg = c // 4, c % 4
        h0 = hg * HPC
        qrows = slice(h0 * HD, (h0 + HPC) * HD)
        o0 = hg * OSL

        wqkT = np.ascontiguousarray(
            np.concatenate([w_deq[0:DIM][qrows],
                            w_deq[DIM:2 * DIM][qrows]], axis=0).T)
        wvT = np.ascontiguousarray(w_deq[2 * DIM:3 * DIM][qrows].T)
        xT = np.ascontiguousarray(x[b].T)

        gains = (q_gain[h0:h0 + HPC] / math.sqrt(HD // 2)).reshape(4, 1)
        featT = np.ascontiguousarray(features[:, o0:o0 + OSL].T)

        p_f = p_deq[o0:o0 + OSL] @ features.T          # [OSL, 16] f32
        p_s = _sigmoid(5.0 * p_f)
        p_a = p_f * p_s
        A_eff = (theta * p_a - alpha * (1.0 - p_s)) / 5.0
        B_eff = -beta * p_a
        m = {
            "xT": xT.astype(np.float32),
            "wqkT": wqkT.astype(np.float32),
            "wvT": wvT.astype(np.float32),
            "cosF": cosF, "sinF": sinF,
            "bigmask": bigmask,
            "gains": gains.astype(np.float32),
            "featT": featT.astype(np.float32),
            "AT": np.ascontiguousarray(A_eff.T).astype(np.float32),
            "BT": np.ascontiguousarray(B_eff.T).astype(np.float32),
        }
        for t in range(3):
            m[f"maskq{t}"] = masks[t]
            m[f"maskk{t}"] = masks[t]
        in_maps.append(m)
    return in_maps


def assemble_output(S, results):
    out = np.empty((B, S, DIM), np.float32)
    for c in range(N_CORES):
        b, hg = c // 4, c % 4
        out[b, :, hg * OSL:(hg + 1) * OSL] = results[c]["out"].T
    return out


_PROGRAM_CACHE = {}


def kernel(x, w_qkv, features, prototypes, theta, alpha, beta, q_gain,
           diff_lambda, _trace=False):
    x = np.asarray(x, np.float32)
    S = x.shape[1]
    lam = np.asarray(diff_lambda, np.float32)
    # lambdas are baked as immediates per local head; all 4 head groups share
    # one program, so they must agree across groups (true for these inputs).
    lam_local = lam.reshape(4, HPC)
    assert np.all(lam_local == lam_local[0:1]), "head-group-varying lambda"

    key = (S, lam_local[0].tobytes())
    if key not in _PROGRAM_CACHE:
        _PROGRAM_CACHE[key] = build_program(S, lam_local[0])
    nc = _PROGRAM_CACHE[key]

    in_maps = make_in_maps(S, x, w_qkv, features, prototypes, theta, alpha,
                           beta, q_gain, diff_lambda)
    res = run_bass_kernel_spmd(nc, in_maps, list(range(N_CORES)),
                               trace=_trace)
    out = assemble_output(S, res.results)
    if _trace:
        return out, res
    return out


# revision 18
# speedup vs baseline: 1.0313x; 1.0313x over previous
"""Trainium2 Bass kernel for nn_CausalSelfAttention_68496138437292.

Sharding: 8 cores = 2 batches x 4 head-groups. Core c handles batch b=c//4 and
heads [4*(c%4), 4*(c%4)+4). The Tversky projection is sharded over out_features
(each core computes a 256-wide o-slice); the feature contraction x_f (summed
over D, which is split across head-groups) uses one small AllReduce over each
batch's 4-core group.

All matmuls run in float32r (full-rate fp32 PE mode). The ternary weight
quantization (bf16 group-wise, matching the reference bit-exactly) and the
RoPE/YaRN tables are precomputed on the host; all heavy math (QKV projection,
rmsnorm stats, rope rotation, causal attention, softmax, Tversky projection)
runs on device.

Layout notes:
- q/k are computed transposed (d on partitions, s free). Matmul operands must
  start at base partition 0/32/64 and DVE ops cannot cross partitions, so the
  8 per-(head,half) 32-row blocks are packed 3-per-tile at offsets {0,32,64};
  projection PSUM is drained by DMA straight into that block layout.
- v lands [s, channel] groups of 33 (32 v columns + a ones column) so the PV
  matmul emits softmax denominators for free in PSUM row 32.
- Scores are computed transposed (S^T[ks, qs]); exp's per-partition ACT scale
  applies the k-side rmsnorm factor, the q-side factor (with q_gain/sqrt(d))
  is multiplied into q after rope via a DMA-broadcast tile.
- All partition-crossing moves (rope half-swap staging, reciprocal broadcast,
  attention-output writeback) go through DMA.
"""

import math
from contextlib import ExitStack

import ml_dtypes
import numpy as np

import concourse.bass as bass
import concourse.mybir as mybir
import concourse.tile as tile
from concourse import bacc
from concourse.bass_utils import run_bass_kernel_spmd

F32 = mybir.dt.float32
F32R = mybir.dt.float32r
AF = mybir.ActivationFunctionType
ALU = mybir.AluOpType

DIM, NH, HD = 1024, 16, 64
ROPE_BASE, TRAIN_LEN, YARN_MAX = 10000.0, 1024, 4096
GROUP = 64
EPS = 1e-05
B = 2
N_CORES = 8
HPC = 4          # heads per core
OSL = 256        # out-feature slice per core


# block layout: (head, half) -> (tile, partition offset); 3 blocks per tile
def _blk(h, f):
    idx = h * 2 + f
    return idx // 3, (idx % 3) * 32


_NBLK = [3, 3, 2]                      # valid blocks per q/k tile
_NROW = [96, 96, 64]                   # valid rows per q/k tile


# ----------------------------------------------------------------- host math

def _ternary_deq(w: np.ndarray) -> np.ndarray:
    """bf16 group-wise ternary dequantized weights; bit-exact with the jax
    reference (mean accumulates in f32, every elementwise op rounds to bf16)."""
    bf = ml_dtypes.bfloat16
    wb = w.astype(bf)
    wg = wb.reshape(-1, GROUP)
    m = (np.sum(np.abs(wg), axis=-1, keepdims=True, dtype=np.float32) / GROUP).astype(bf)
    scale = np.maximum(m.astype(np.float32), np.float32(1e-8)).astype(bf)
    ratio = (wg.astype(np.float32) / scale.astype(np.float32)).astype(bf)
    q = np.clip(np.round(ratio.astype(np.float32)), -1.0, 1.0).astype(bf)
    deq = (q.astype(np.float32) * scale.astype(np.float32)).astype(bf)
    return deq.reshape(wb.shape).astype(np.float32)


def _rope_tables(seqlen: int):
    rd = HD
    ar = np.arange(0, rd, 2, dtype=np.float32)
    inv_freq = 1.0 / ROPE_BASE ** (ar / rd)
    scale = TRAIN_LEN / YARN_MAX
    ramp = np.clip((ar / rd - 0.25) / 0.75, 0.0, 1.0)
    inv_freq = inv_freq / (ramp * (1.0 / scale - 1.0) + 1.0)
    freqs = np.arange(seqlen, dtype=np.float32)[:, None] * inv_freq[None, :]
    # [S, 32] -> [32, S]
    return np.cos(freqs).T.astype(np.float32), np.sin(freqs).T.astype(np.float32)


def _sigmoid(x):
    return 1.0 / (1.0 + np.exp(-x))


# ------------------------------------------------------------ device program

def build_program(S: int, lam: np.ndarray, dbg: bool = False):
    """Build the SPMD Bass program. lam: [HPC] per-local-head diff_lambda
    (baked as immediates; must be identical across head groups)."""
    NT = S // 128          # s-tiles
    NJ = S // 512          # 512-wide qs chunks
    SQ = S // 4            # projection s-quarter width
    NTQ = SQ // 128        # s-tiles per quarter

    nc = bacc.Bacc("TRN2", target_bir_lowering=False, debug=False,
                   num_devices=N_CORES)

    # DRAM I/O (per-core contents differ via in_maps)
    d_xT = nc.dram_tensor("xT", [DIM, S], F32R, kind="ExternalInput")
    d_wqkT = nc.dram_tensor("wqkT", [DIM, 512], F32R, kind="ExternalInput")
    d_wvT = nc.dram_tensor("wvT", [DIM, 256], F32R, kind="ExternalInput")
    d_cosF = nc.dram_tensor("cosF", [128, S], F32, kind="ExternalInput")
    d_sinF = nc.dram_tensor("sinF", [128, S], F32, kind="ExternalInput")
    d_bigmask = nc.dram_tensor("bigmask", [128, 896], F32, kind="ExternalInput")
    d_maskq = [nc.dram_tensor(f"maskq{t}", [128, 4], F32R, kind="ExternalInput")
               for t in range(3)]
    d_maskk = [nc.dram_tensor(f"maskk{t}", [128, 4], F32R, kind="ExternalInput")
               for t in range(3)]
    d_gains = nc.dram_tensor("gains", [4, 1], F32, kind="ExternalInput")
    d_featT = nc.dram_tensor("featT", [OSL, 16], F32R, kind="ExternalInput")
    d_AT = nc.dram_tensor("AT", [16, OSL], F32R, kind="ExternalInput")
    d_BT = nc.dram_tensor("BT", [16, OSL], F32R, kind="ExternalInput")
    d_out = nc.dram_tensor("out", [OSL, S], F32, kind="ExternalOutput")
    if dbg:
        d_dbg_qa = [nc.dram_tensor(f"dbg_qa{t}", [128, S], F32, kind="ExternalOutput")
                    for t in range(3)]
        d_dbg_ka = [nc.dram_tensor(f"dbg_ka{t}", [128, S], F32, kind="ExternalOutput")
                    for t in range(3)]
        d_dbg_v = nc.dram_tensor("dbg_v0", [128, 264], F32, kind="ExternalOutput")
        d_dbg_invT = nc.dram_tensor("dbg_invT", [128, 4 * (S // 128)], F32,
                                    kind="ExternalOutput")
        d_dbg_inva = nc.dram_tensor("dbg_inva", [4, S], F32, kind="ExternalOutput")
        d_dbg_yT = [nc.dram_tensor(f"dbg_yT{i}", [128, S], F32, kind="ExternalOutput")
                    for i in range(2)]
        d_dbg_xf = nc.dram_tensor("dbg_xf", [16, S], F32, kind="ExternalOutput")

    with tile.TileContext(nc) as tc:
        persist = ExitStack()
        cpool = persist.enter_context(tc.tile_pool(name="consts", bufs=1))
        vpool = persist.enter_context(tc.tile_pool(name="vstore", bufs=1))
        ivpool = persist.enter_context(tc.tile_pool(name="invT", bufs=1))
        ypool = persist.enter_context(tc.tile_pool(name="yT", bufs=1))
        qkpool = persist.enter_context(tc.tile_pool(name="qk", bufs=1))
        drpool = persist.enter_context(
            tc.tile_pool(name="drscratch", bufs=1, space="DRAM"))

        # small constants (live for the whole kernel)
        bigmask = cpool.tile([128, 896], F32, name="bigmask")
        maskq = [cpool.tile([128, 4], F32R, name=f"maskq{t}") for t in range(3)]
        maskk = [cpool.tile([128, 4], F32R, name=f"maskk{t}") for t in range(3)]
        gains = cpool.tile([4, 1], F32, name="gains")
        eps_col = cpool.tile([128, 1], F32, name="eps")
        featT = [cpool.tile([128, 16], F32R, name=f"featT{i}") for i in range(2)]
        AT = cpool.tile([16, OSL], F32R, name="AT")
        BT = cpool.tile([16, OSL], F32R, name="BT")
        nc.vector.memset(eps_col[:], EPS)
        nc.sync.dma_start(bigmask[:], d_bigmask[:])
        for t in range(3):
            nc.sync.dma_start(maskq[t][:], d_maskq[t][:])
            nc.sync.dma_start(maskk[t][:], d_maskk[t][:])
        nc.sync.dma_start(gains[:], d_gains[:])
        nc.sync.dma_start(featT[0][:], d_featT[0:128, :])
        nc.sync.dma_start(featT[1][:], d_featT[128:256, :])
        nc.sync.dma_start(AT[:], d_AT[:])
        nc.sync.dma_start(BT[:], d_BT[:])

        # v storage: per s-tile, 8 groups of (32 v-cols + ones col)
        v_store = [vpool.tile([128, 264], F32R, name=f"v{st}") for st in range(NT)]
        invT = ivpool.tile([128, 4 * NT], F32, name="invT")
        yT = [ypool.tile([128, S], F32R, name=f"yT{i}") for i in range(2)]
        # q/k in block layout; rope happens in place on these tiles
        qa = [qkpool.tile([128, S], F32R, name=f"qa{t}") for t in range(3)]
        ka = [qkpool.tile([128, S], F32R, name=f"ka{t}") for t in range(3)]

        # ---------------- phase 1: QKV projection ----------------
        proj_scope = ExitStack()
        wpool = proj_scope.enter_context(tc.tile_pool(name="weights", bufs=1))
        xpool = proj_scope.enter_context(tc.tile_pool(name="xstream", bufs=3))
        psq = proj_scope.enter_context(
            tc.tile_pool(name="psq", bufs=1, space="PSUM"))
        psv = proj_scope.enter_context(
            tc.tile_pool(name="psv", bufs=1, space="PSUM"))

        wqk = [wpool.tile([128, 512], F32R, name=f"wqk{d}") for d in range(8)]
        wv = [wpool.tile([128, 256], F32R, name=f"wv{d}") for d in range(8)]
        for d in range(8):
            nc.sync.dma_start(wqk[d][:], d_wqkT[d * 128:(d + 1) * 128, :])
            nc.sync.dma_start(wv[d][:], d_wvT[d * 128:(d + 1) * 128, :])

        for q4 in range(4):
            s0 = q4 * SQ
            pq = [psq.tile([128, SQ], F32, tag=f"pq{ch}", name=f"pq{ch}")
                  for ch in range(4)]
            pv = [psv.tile([128, 256], F32, tag=f"pv{st}", name=f"pv{st}")
                  for st in range(NTQ)]
            for d in range(8):
                xt = xpool.tile([128, SQ], F32R, tag="xt", name="xt")
                nc.sync.dma_start(xt[:], d_xT[d * 128:(d + 1) * 128, s0:s0 + SQ])
                for ch in range(4):
                    nc.tensor.matmul(
                        pq[ch][:], wqk[d][:, ch * 128:(ch + 1) * 128], xt[:],
                        start=(d == 0), stop=(d == 7))
                for st in range(NTQ):
                    nc.tensor.matmul(
                        pv[st][:], xt[:, st * 128:(st + 1) * 128], wv[d][:],
                        start=(d == 0), stop=(d == 7))
            # qk drain: ACT copy psum -> aligned SBUF temp, then DMA each
            # (head, half) 32-row block into the packed block layout.
            # psum ch-tile layout: 2 heads x 64.
            for ch in range(4):
                tmpd = xpool.tile([128, SQ], F32R, tag="tmpd", name="tmpd",
                                  bufs=4)
                nc.scalar.activation(tmpd[:], pq[ch][:], AF.Copy)
                dst_tiles = qa if ch < 2 else ka
                for hl in range(2):          # head within ch-tile
                    h = (ch % 2) * 2 + hl
                    for f in range(2):
                        t, o = _blk(h, f)
                        nc.sync.dma_start(
                            dst_tiles[t][o:o + 32, s0:s0 + SQ],
                            tmpd[hl * 64 + f * 32:hl * 64 + f * 32 + 32, :])
            for st in range(NTQ):
                gst = q4 * NTQ + st
                nc.vector.memset(v_store[gst][:].bitcast(F32), 1.0)
                dst = v_store[gst][:].rearrange("p (g c) -> p g c", c=33)[:, :, 0:32]
                src = pv[st][:].rearrange("p (g c) -> p g c", c=32)
                nc.scalar.activation(dst, src, AF.Copy)
        proj_scope.close()

        # ---------------- phase 2: rmsnorm stats ----------------
        qsc_scope = ExitStack()
        qsc_pool = qsc_scope.enter_context(tc.tile_pool(name="qscale", bufs=1))
        norm_scope = ExitStack()
        sqpool = norm_scope.enter_context(tc.tile_pool(name="sq", bufs=1))
        inva_pool = norm_scope.enter_context(tc.tile_pool(name="inva", bufs=1))
        psa = norm_scope.enter_context(
            tc.tile_pool(name="psa", bufs=1, space="PSUM"))
        psb = norm_scope.enter_context(
            tc.tile_pool(name="psb", bufs=3, space="PSUM"))

        # q-side: sumsq per (head, s) as [4, S]
        pa = psa.tile([4, S], F32, name="pa")
        for t in range(3):
            n = _NROW[t]
            sq = sqpool.tile([128, S], F32R, tag=f"sq{t}", name=f"sq{t}")
            nc.scalar.activation(sq[0:n, :], qa[t][0:n, :], AF.Square)
            for qc in range(S // 512):
                nc.tensor.matmul(
                    pa[:, qc * 512:(qc + 1) * 512], maskq[t][0:n, :],
                    sq[0:n, qc * 512:(qc + 1) * 512],
                    start=(t == 0), stop=(t == 2))
        inv_a = inva_pool.tile([4, S], F32, name="inva")
        nc.scalar.activation(inv_a[:], pa[:], AF.Sqrt, scale=1.0 / HD,
                             bias=eps_col[0:4, :])
        nc.vector.reciprocal(inv_a[:], inv_a[:])
        nc.vector.tensor_scalar_mul(inv_a[:], inv_a[:], gains[:])

        if dbg:
            nc.sync.dma_start(d_dbg_inva[:], inv_a[:])
        # broadcast inv_a rows into q-block layout via DRAM round trip
        dr_inva = drpool.tile([4, S], F32, name="dr_inva")
        nc.sync.dma_start(dr_inva[:], inv_a[:])
        qsc = [qsc_pool.tile([128, S], F32, name=f"qsc{t}") for t in range(3)]
        for h in range(4):
            for f in range(2):
                t, o = _blk(h, f)
                nc.sync.dma_start(
                    qsc[t][o:o + 32, :],
                    dr_inva[h:h + 1, :].to_broadcast([32, S]))

        # k-side: sumsq transposed as [s-tile, 4] columns of invT
        sq_k = []
        for t in range(3):
            n = _NROW[t]
            sq = sqpool.tile([128, S], F32R, tag=f"sqk{t}", name=f"sqk{t}")
            nc.scalar.activation(sq[0:n, :], ka[t][0:n, :], AF.Square)
            sq_k.append(sq)
        for st in range(NT):
            pb = psb.tile([128, 4], F32, tag="pb", name="pb")
            for t in range(3):
                n = _NROW[t]
                nc.tensor.matmul(
                    pb[:], sq_k[t][0:n, st * 128:(st + 1) * 128], maskk[t][0:n, :],
                    start=(t == 0), stop=(t == 2))
            nc.scalar.activation(invT[:, st * 4:st * 4 + 4], pb[:], AF.Sqrt,
                                 scale=1.0 / HD, bias=eps_col[:])
        nc.vector.reciprocal(invT[:], invT[:])
        norm_scope.close()

        # -------------- phase 3: rope (+ q scaling), in place --------------
        rope_scope = ExitStack()
        tripool = rope_scope.enter_context(tc.tile_pool(name="trig", bufs=1))
        rp = rope_scope.enter_context(tc.tile_pool(name="ropetmp", bufs=2))

        cosF = tripool.tile([128, S], F32, name="cosF")
        sinF = tripool.tile([128, S], F32, name="sinF")
        nc.sync.dma_start(cosF[:], d_cosF[:])
        nc.sync.dma_start(sinF[:], d_sinF[:])

        def rope_all(tiles, qscale_tiles):
            # Prefetch ALL partner copies (other half of each head) before the
            # in-place overwrites: partner pairs can span tiles (h1's halves
            # live in tiles 0 and 1).
            prts = []
            for t in range(3):
                prt = rp.tile([128, S], F32R, tag=f"prt{t}", name=f"prt{t}")
                for k in range(_NBLK[t]):
                    idx = 3 * t + k
                    h, f = idx // 2, idx % 2
                    pt, po = _blk(h, 1 - f)
                    nc.sync.dma_start(prt[k * 32:k * 32 + 32, :],
                                      tiles[pt][po:po + 32, :])
                prts.append(prt)
            # rot_f0 = x0*cos + x1*sin ; rot_f1 = x1*cos - x0*sin.
            # sinF rows carry sign (+,-,+) per 32-block, matching the f-parity
            # of blocks in tiles 0 and 2; tile 1 has parity (-,+,-) so the
            # sign flips there -> subtract instead of add.
            for t in range(3):
                n = _NROW[t]
                tb = rp.tile([128, S], F32, tag="tb", name="tb")
                nc.vector.tensor_mul(tb[0:n, :], prts[t][0:n, :], sinF[0:n, :])
                nc.vector.tensor_mul(tiles[t][0:n, :], tiles[t][0:n, :],
                                     cosF[0:n, :])
                if t == 1:
                    nc.vector.tensor_sub(tiles[t][0:n, :], tiles[t][0:n, :],
                                         tb[0:n, :])
                else:
                    nc.vector.tensor_add(tiles[t][0:n, :], tiles[t][0:n, :],
                                         tb[0:n, :])
                if qscale_tiles is not None:
                    nc.vector.tensor_mul(tiles[t][0:n, :], tiles[t][0:n, :],
                                         qscale_tiles[t][0:n, :])

        rope_all(qa, qsc)
        rope_all(ka, None)
        rope_scope.close()
        qsc_scope.close()
        if dbg:
            for t in range(3):
                nc.sync.dma_start(d_dbg_qa[t][:], qa[t][:].bitcast(F32))
                nc.sync.dma_start(d_dbg_ka[t][:], ka[t][:].bitcast(F32))
            nc.sync.dma_start(d_dbg_v[:], v_store[0][:].bitcast(F32))
            nc.sync.dma_start(d_dbg_invT[:], invT[:])

        # ---------------- phase 4: attention ----------------
        attn_scope = ExitStack()
        epool = attn_scope.enter_context(tc.tile_pool(name="expS", bufs=18))
        tpool = attn_scope.enter_context(tc.tile_pool(name="exptmp", bufs=4))
        zpool = attn_scope.enter_context(tc.tile_pool(name="zc", bufs=6))
        rcpool = attn_scope.enter_context(tc.tile_pool(name="rcp", bufs=4))
        bcpool = attn_scope.enter_context(tc.tile_pool(name="bcast", bufs=4))
        pss = attn_scope.enter_context(
            tc.tile_pool(name="pss", bufs=5, space="PSUM"))
        psy = attn_scope.enter_context(
            tc.tile_pool(name="psy", bufs=3, space="PSUM"))

        for h in range(HPC):
            for j in range(NJ):
                ntk = 4 * (j + 1)
                LAG = 5
                es = {}
                py = {}
                for f in (0, 1):
                    py[f] = psy.tile([33, 512], F32, tag="py", name="py")

                def emit_pv(f, t):
                    g = 2 * h + f
                    nc.tensor.matmul(
                        py[f][:], v_store[t][:, g * 33:g * 33 + 33],
                        es.pop((f, t))[:], start=(t == 0), stop=(t == ntk - 1))

                for t in range(ntk):
                    for f in (0, 1):
                        qt, qo = _blk(h, f)
                        qr = qa[qt][qo:qo + 32, j * 512:(j + 1) * 512]
                        ps = pss.tile([128, 512], F32, tag="ps", name="ps")
                        nc.tensor.matmul(
                            ps[:],
                            ka[qt][qo:qo + 32, t * 128:(t + 1) * 128],
                            qr, start=True, stop=True)
                        sc = invT[:, t * 4 + h:t * 4 + h + 1]
                        et = epool.tile([128, 512], F32R, tag="e", name="e")
                        if t >= 4 * j:  # causal boundary tile
                            off = (t - 4 * j) * 128
                            tmp = tpool.tile([128, 512], F32, tag="tmp",
                                             name="tmp")
                            nc.scalar.activation(tmp[:], ps[:], AF.Exp, scale=sc)
                            nc.vector.tensor_mul(
                                et[:], tmp[:],
                                bigmask[:, 384 - off:896 - off])
                        else:
                            nc.scalar.activation(et[:], ps[:], AF.Exp, scale=sc)
                        es[(f, t)] = et
                    if t >= LAG:
                        for f in (0, 1):
                            emit_pv(f, t - LAG)
                for t in range(max(0, ntk - LAG), ntk):
                    for f in (0, 1):
                        emit_pv(f, t)
                # combine halves (all at base partition 0, then DMA into yT)
                bb = {}
                for f in (0, 1):
                    rc = rcpool.tile([33, 512], F32, tag="rc", name="rc")
                    nc.vector.reciprocal(rc[32:33, :], py[f][32:33, :])
                    dr_rc = drpool.tile([1, 512], F32, tag="drrc", bufs=4,
                                        name="drrc")
                    nc.gpsimd.dma_start(dr_rc[:], rc[32:33, :])
                    bc = bcpool.tile([32, 512], F32, tag="bc", name="bc")
                    nc.gpsimd.dma_start(bc[:], dr_rc[:].to_broadcast([32, 512]))
                    bb[f] = bc
                z1 = zpool.tile([32, 512], F32, tag="z", name="z")
                z2 = zpool.tile([32, 512], F32, tag="z", name="z")
                zo1 = zpool.tile([32, 512], F32, tag="z", name="z")
                zo2 = zpool.tile([32, 512], F32, tag="z", name="z")
                nc.vector.tensor_mul(z1[:], py[0][0:32, :], bb[0][:])
                nc.vector.tensor_mul(z2[:], py[1][0:32, :], bb[1][:])
                lam_h = float(lam[h])
                nc.vector.scalar_tensor_tensor(
                    zo1[:], z2[:], -lam_h, z1[:], ALU.mult, ALU.add)
                nc.vector.scalar_tensor_tensor(
                    zo2[:], z2[:], lam_h, z1[:], ALU.mult, ALU.add)
                ti, r0 = h // 2, (h % 2) * 64
                nc.gpsimd.dma_start(
                    yT[ti][r0:r0 + 32, j * 512:(j + 1) * 512],
                    zo1[:].bitcast(F32R))
                nc.gpsimd.dma_start(
                    yT[ti][r0 + 32:r0 + 64, j * 512:(j + 1) * 512],
                    zo2[:].bitcast(F32R))
        attn_scope.close()

        # ---------------- phase 5: tversky projection ----------------
        tv_scope = ExitStack()
        xfpool = tv_scope.enter_context(tc.tile_pool(name="xf", bufs=1))
        psxf = tv_scope.enter_context(
            tc.tile_pool(name="psxf", bufs=1, space="PSUM"))

        pxf = psxf.tile([16, S], F32, name="pxf")
        for qc in range(S // 512):
            for dc in range(2):
                nc.tensor.matmul(
                    pxf[:, qc * 512:(qc + 1) * 512], featT[dc][:],
                    yT[dc][:, qc * 512:(qc + 1) * 512],
                    start=(dc == 0), stop=(dc == 1))
        xf_loc = xfpool.tile([16, S], F32, name="xfl")
        nc.scalar.activation(xf_loc[:], pxf[:], AF.Copy)

        cc_in = drpool.tile([16, S], F32, name="ccin")
        cc_out = drpool.tile([16, S], F32, name="ccout")
        nc.sync.dma_start(cc_in[:], xf_loc[:])
        nc.gpsimd.collective_compute(
            "AllReduce", ALU.add,
            replica_groups=[[0, 1, 2, 3], [4, 5, 6, 7]],
            ins=[cc_in[:]], outs=[cc_out[:]])
        xf = xfpool.tile([16, S], F32, name="xfr")
        nc.sync.dma_start(xf[:], cc_out[:])
        if dbg:
            for i in range(2):
                nc.sync.dma_start(d_dbg_yT[i][:], yT[i][:].bitcast(F32))
            nc.sync.dma_start(d_dbg_xf[:], xf[:])

        xa = xfpool.tile([16, S], F32R, name="xa")
        oms = xfpool.tile([16, S], F32R, name="oms")
        nc.scalar.activation(xa[:], xf[:], AF.Silu, scale=5.0)
        nc.scalar.activation(oms[:], xf[:], AF.Sigmoid, scale=-5.0)

        tv2_scope = ExitStack()
        opool = tv2_scope.enter_context(tc.tile_pool(name="outsb", bufs=1))
        pso = tv2_scope.enter_context(
            tc.tile_pool(name="pso", bufs=2, space="PSUM"))

        out_sb = [opool.tile([128, S], F32, name=f"osb{i}") for i in range(2)]
        for ot in range(2):
            for qc in range(S // 512):
                po = pso.tile([128, 512], F32, tag="po", name="po")
                nc.tensor.matmul(
                    po[:], AT[:, ot * 128:(ot + 1) * 128],
                    xa[:, qc * 512:(qc + 1) * 512], start=True, stop=False)
                nc.tensor.matmul(
                    po[:], BT[:, ot * 128:(ot + 1) * 128],
                    oms[:, qc * 512:(qc + 1) * 512], start=False, stop=True)
                nc.scalar.activation(
                    out_sb[ot][:, qc * 512:(qc + 1) * 512], po[:], AF.Copy)
            nc.sync.dma_start(d_out[ot * 128:(ot + 1) * 128, :], out_sb[ot][:])
        tv2_scope.close()
        tv_scope.close()
        persist.close()

    nc.compile()
    return nc


# ----------------------------------------------------------- host marshaling

def make_in_maps(S, x, w_qkv, features, prototypes, theta, alpha, beta,
                 q_gain, diff_lambda):
    x = np.asarray(x, np.float32)
    w_qkv = np.asarray(w_qkv, np.float32)
    features = np.asarray(features, np.float32)
    prototypes = np.asarray(prototypes, np.float32)
    theta = float(np.abs(np.asarray(theta, np.float32)))
    alpha = float(np.abs(np.asarray(alpha, np.float32)))
    beta = float(np.abs(np.asarray(beta, np.float32)))
    q_gain = np.asarray(q_gain, np.float32)

    w_deq = _ternary_deq(w_qkv)
    p_deq = _ternary_deq(prototypes)
    cosT, sinT = _rope_tables(S)       # [32, S] each

    rows = np.arange(128)
    # rope tables in block layout: row r belongs to block r//32 with d = r%32.
    # sinF carries the sign of the f-parity pattern of tiles 0/2 (+,-,+);
    # tile 1's pattern (-,+,-) is realized by subtracting instead of adding.
    cosF = cosT[rows % 32, :]
    sgn = np.where((rows // 32) % 2 == 0, 1.0, -1.0).astype(np.float32)
    sinF = sinT[rows % 32, :] * sgn[:, None]

    bigmask = (np.arange(896)[None, :] >= rows[:, None] + 384).astype(np.float32)

    # norm masks in block layout: block k of tile t belongs to head (3t+k)//2
    masks = []
    for t in range(3):
        m = np.zeros((128, 4), np.float32)
        for k in range(_NBLK[t]):
            head = (3 * t + k) // 2
            m[k * 32:(k + 1) * 32, head] = 1.0
        masks.append(m)

    in_maps = []
    for c in range(N_CORES):
        b, hg = c // 4, c % 4
        h0 = hg * HPC
        qrows = slice(h0 * HD, (h0 + HPC) * HD)
        o0 = hg * OSL

        wqkT = np.ascontiguousarray(
            np.concatenate([w_deq[0:DIM][qrows],
                            w_deq[DIM:2 * DIM][qrows]], axis=0).T)
        wvT = np.ascontiguousarray(w_deq[2 * DIM:3 * DIM][qrows].T)
        xT = np.ascontiguousarray(x[b].T)

        gains = (q_gain[h0:h0 + HPC] / math.sqrt(HD // 2)).reshape(4, 1)
        featT = np.ascontiguousarray(features[:, o0:o0 + OSL].T)

        p_f = p_deq[o0:o0 + OSL] @ features.T          # [OSL, 16] f32
        p_s = _sigmoid(5.0 * p_f)
        p_a = p_f * p_s
        A_eff = (theta * p_a - alpha * (1.0 - p_s)) / 5.0
        B_eff = -beta * p_a
        m = {
            "xT": xT.astype(np.float32),
            "wqkT": wqkT.astype(np.float32),
            "wvT": wvT.astype(np.float32),
            "cosF": cosF, "sinF": sinF,
            "bigmask": bigmask,
            "gains": gains.astype(np.float32),
            "featT": featT.astype(np.float32),
            "AT": np.ascontiguousarray(A_eff.T).astype(np.float32),
            "BT": np.ascontiguousarray(B_eff.T).astype(np.float32),
        }
        for t in range(3):
            m[f"maskq{t}"] = masks[t]
            m[f"maskk{t}"] = masks[t]
        in_maps.append(m)
    return in_maps


def assemble_output(S, results):
    out = np.empty((B, S, DIM), np.float32)
    for c in range(N_CORES):
        b, hg = c // 4, c % 4
        out[b, :, hg * OSL:(hg + 1) * OSL] = results[c]["out"].T
    return out


_PROGRAM_CACHE = {}


def kernel(x, w_qkv, features, prototypes, theta, alpha, beta, q_gain,
           diff_lambda, _trace=False):
    x = np.asarray(x, np.float32)
    S = x.shape[1]
    lam = np.asarray(diff_lambda, np.float32)
    # lambdas are baked as immediates per local head; all 4 head groups share
    # one program, so they must agree across groups (true for these inputs).
    lam_local = lam.reshape(4, HPC)
    assert np.all(lam_local == lam_local[0:1]), "head-group-varying lambda"

    key = (S, lam_local[0].tobytes())
    if key not in _PROGRAM_CACHE:
        _PROGRAM_CACHE[key] = build_program(S, lam_local[0])
    nc = _PROGRAM_CACHE[key]

    in_maps = make_in_maps(S, x, w_qkv, features, prototypes, theta, alpha,
                           beta, q_gain, diff_lambda)
    res = run_bass_kernel_spmd(nc, in_maps, list(range(N_CORES)),
                               trace=_trace)
    out = assemble_output(S, res.results)
    if _trace:
        return out, res
    return out
